# revision 31
# baseline (speedup 1.0000x reference)
"""DGI (Deep Graph Infomax) forward kernel for 8 TRN2 NeuronCores.

Problem (all shapes hardcoded):
  seq1, seq2: [1, 8192, 128] f32   node features
  adj:        [1, 8192, 8192] f32  dense adjacency
  cc_label:   [8, 1024] i32        community partition (arange layout)
  W: [128,128], b: [128], Wb: [128,128], bb: [] f32
  out:        [1, 16384] f32       = concat(ret1, ret2)

Math per GCN branch: h = relu(adj @ (seq @ W) + b), reassociated to
(adj @ seq) @ W so the big contraction uses seq tiles as the stationary
operand and a host-transposed adj block as the moving operand. Everything
lives in "transposed" space (features on partitions): the community mean
is a free-axis accumulation and the bilinear scores are 1-column matmuls.

Sharding: core k owns nodes [1024k, 1024k+1024) == community k (cc_label
is arange). No collectives.

Data layout (the whole point of this version): seq tile t is consumed by
exactly the two matmuls of m-tile t, so the host packs, per partition p
and tile t, [seq1 row | seq2 row | adjT rows] into one combined tensor
comb[128, 64, 1280] f16 (2560 B per partition-tile). One DMA stream in
tile order IS the consumption order, every transfer has >=5 KB contiguous
per-partition runs (small runs were the previous bottleneck: 512 B
packets move at ~20 GB/s/engine vs 4 KB at full rate), and no persistent
seq buffer is needed.

Schedule:
  - sync HWDGE queue: comb[0:2][2:4][4:6][6:8][8:12][12:16] into a warm
    tile (fine-grained so the PE starts as early as possible), then
    8-tile groups [16:24][32:40][48:56][56:64] from a 3-buffer pool.
  - scalar HWDGE queue: params, then groups [24:32][40:48] (it starts
    ~1 us later than sync; interleaving two queues overlaps the
    small-packet warmup with steady streaming).
  - All small matmuls (W-contraction, cw, scores) in fp16. adj is
    pre-scaled by 256 (fp16 range); the relu computes h'=relu(z+256b)
    and the 1/256 is folded into host-side wbt and the sigmoid scale.
  - m-dim split in halves of 32 tiles; half-1's W-contraction + copies
    run mid-stream. For the last 16 m-tiles, branch 0 (which gates
    sigmoid -> cw -> all scores) streams before branch 1, and branch 0's
    epilogue PE ops are interleaved between branch-1 matmul batches (the
    PE queue is in-order). Branch 1 finishes c-major over the last 3
    tiles so chunk 0's epilogue overlaps chunk 1's matmuls.
  - Output via a single DMA on the sync queue.
"""

import numpy as np

import concourse.bass as bass
import concourse.tile as tile
from concourse import bacc, mybir
from concourse.bass_utils import run_bass_kernel_spmd

N = 8192          # nodes
D = 128           # input feature dim
H = 128           # hidden dim
NC = 8            # communities / cores
CS = N // NC      # community size (nodes per core)
MT = N // 128     # number of 128-row m-tiles (64)
HALF = MT // 2
CHUNK = 512       # matmul moving free dim (psum bank width in fp32)
NCH = CS // CHUNK # n-chunks per core (2)

F32 = mybir.dt.float32
F16 = mybir.dt.float16
ADJ_SCALE = 256.0  # keeps fp16(adj*scale) in the normal range; undone via
                   # host-prescaled wbt (scores) and the sigmoid scale (mean)

ROW = 2 * D + CS   # per-partition-tile row: seq1 | seq2 | adjT = 1280 f16
AOFF = 2 * D       # adj offset within a row

# Stream/processing architecture (lessons from five traced schedules):
#  - The main stream rides ONE HWDGE queue (sync), strictly in consumption
#    order. Two queues racing over the stream starve the PE: HBM bandwidth
#    goes to whichever queue has descriptors in flight, so the queue
#    holding the PE's next tile can lag many tiles behind.
#  - The m-tiles processed LAST (the "tail", where branch 0 finishes early
#    to gate sigmoid -> cw -> scores) are remapped to tiles 2..13, which
#    the scalar queue delivers during the ramp. The dual-queue ramp is
#    additive (the early bottleneck is per-queue descriptor pipelines, not
#    HBM), the scalar queue stops pulling by ~16 us (no mid-stream race),
#    and the 2x-rate tail sweep plus the whole epilogue runs out of SBUF -
#    also insulating the finish from stream-end straggler DMA engines
#    (observed: one engine finishing its packet backlog 10 us after the
#    other 15, gating the last group's completion semaphore).
#  - Tile 0 is split at the chunk-0/chunk-1 adj boundary so the first
#    matmuls start before its chunk-1 columns land; the next few stream
#    ranges stay small to bound PE wait quantization during the ramp.
MAIN = list(range(50))                 # streamed + processed in order
TAIL = list(range(50, MT))             # 14 m-tiles processed last
SYNC_RANGES = [(1, 2), (2, 3), (3, 4), (4, 6), (6, 8), (8, 10), (10, 12),
               (12, 14), (14, 16), (16, 18), (18, 20), (20, 22), (22, 24),
               (24, 26), (26, 28), (28, 30), (30, 32), (32, 34), (34, 36),
               (36, 38), (38, 40), (40, 42), (42, 44), (44, 46), (46, 48),
               (48, 50), (50, 52), (52, 54), (54, 56), (56, 58), (58, 60),
               (60, 62), (62, 63), (63, 64)]
SCALAR_RANGES = []
PROC = {t: i for i, t in enumerate(MAIN + TAIL)}  # processing index

# PE clock warmup: the HAM clock gate holds the PE at 1.2 GHz until it has
# been busy ~3.4 us, so the first real matmuls would otherwise run at half
# rate. While the first comb tiles are still in flight, run dummy 256-col
# matmuls on a memset scratch tile (no DMA dependency): they start right
# after the engine preamble (~7.2 us) and warm the clock to 2.4 GHz by the
# time real data lands. 256 cols cold = ~213 ns each, so a late-arriving
# real matmul is delayed by at most one dummy.
N_WARM_MM = 13

def _build_module() -> bass.Bass:
    nc = bacc.Bacc()

    comb = nc.declare_dram_parameter("comb", [128, MT, ROW], F16, isOutput=False)
    wq = nc.declare_dram_parameter("wq", [D, 2 * H], F16, isOutput=False)
    bvec = nc.declare_dram_parameter("bvec", [H, 1], F32, isOutput=False)
    out = nc.declare_dram_parameter("out", [2, CS], F32, isOutput=True)

    with tile.TileContext(nc) as tc:
        _emit(tc, comb, wq, bvec, out)
    nc.finalize()
    return nc


def _emit(tc, comb, wq, bvec, out):
    nc = tc.nc
    AF = mybir.ActivationFunctionType
    with (
        tc.tile_pool(name="singles", bufs=1) as singles,
        tc.tile_pool(name="adj_pool", bufs=1) as adj_pool,
        tc.tile_pool(name="misc", bufs=1) as misc,
        tc.tile_pool(name="psum", bufs=1, space="PSUM") as psum,
    ):
        # ---- Params on the scalar HWDGE queue; warmup scratch via gpsimd.
        scratch = singles.tile([128, 256], F16)
        nc.gpsimd.memset(scratch, 0.0)
        # Trigger the lazy sigmoid ACT_TABLE_LOAD (table_sel=1, ~1.3 us)
        # now, in the startup shadow - at first real use it lands on the
        # sigmoid -> cw -> scores critical path and idles the PE long
        # enough to re-throttle its clock.
        sig_warm = singles.tile([1, 1], F32)
        nc.scalar.activation(out=sig_warm, in_=scratch[:1, :1], func=AF.Sigmoid)
        wq_sb = singles.tile([D, 2 * H], F16)
        nc.scalar.dma_start(out=wq_sb, in_=wq[:])
        b_sb = singles.tile([H, 1], F32)
        nc.scalar.dma_start(out=b_sb, in_=bvec[:])
        w_sb = wq_sb[:, 0:H]
        wbt_sb = wq_sb[:, H:2 * H]

        # Every range gets its own buffer: with a rotating pool the DMA of
        # group i+k gates on consumption of group i, which lock-steps the
        # stream behind the PE. 64 tiles of comb fit in SBUF outright.
        # Tile 0 is split at the chunk-0/chunk-1 adj boundary (two separate
        # tiles so the chunk-0 matmuls wait only on the first transfer).
        t0a = adj_pool.tile([128, AOFF + CHUNK], F16)
        nc.sync.dma_start(out=t0a, in_=comb[:, 0, :AOFF + CHUNK])
        t0b = adj_pool.tile([128, CHUNK], F16)
        nc.sync.dma_start(out=t0b, in_=comb[:, 0, AOFF + CHUNK:])
        adj_bufs = {}
        for q_ranges, eng in ((SYNC_RANGES, nc.sync), (SCALAR_RANGES, nc.scalar)):
            for t0, t1 in q_ranges:
                gn = t1 - t0
                buf = adj_pool.tile([128, gn, ROW], F16, name=f"adj_g{t0}")
                eng.dma_start(out=buf, in_=comb[:, t0:t1])
                for u in range(gn):
                    adj_bufs[t0 + u] = (buf, u)

        # ---- Tiles.
        z = [
            [
                [psum.tile([128, CHUNK], F32, name=f"z_{h}_{s}_{c}") for c in range(NCH)]
                for s in range(2)
            ]
            for h in range(2)
        ]
        # zt is reused across the two m-halves (WAR dep on the half-1
        # W-contraction orders the half-2 copies automatically)
        zt_sc = [
            [misc.tile([128, CHUNK], F16, name=f"zt_{s}_{c}") for c in range(NCH)]
            for s in range(2)
        ]
        zt = [zt_sc, zt_sc]
        # h reuses the zt buffers: zt[s][c]'s last reader is the half-2
        # W-contraction, which precedes the relu that writes h (WAR dep)
        h_sb = zt_sc
        csum = [misc.tile([H, 1], F32, name=f"csum_{c}") for c in range(NCH)]
        csum_tot = misc.tile([H, 1], F32)
        c_sb = misc.tile([H, 1], F16)
        cw_sb = misc.tile([H, 1], F16)
        out_sb = misc.tile([1, 2, CS], F32)

        # ---- PE clock warmup (see N_WARM_MM comment). Writes land in a
        # psum bank whose first real matmul is late in the stream and uses
        # start=True, so the garbage is overwritten.
        for _ in range(N_WARM_MM):
            nc.tensor.matmul(
                z[1][1][1][:1, :256], scratch[:, :1], scratch, start=True, stop=True
            )

        def dummy_mm(n=1):
            # HAM-insurance filler between early real tiles: keeps the PE
            # busy through short supply hiccups so the clock gate never
            # re-throttles; costs 213 ns each if data was on time.
            for _ in range(n):
                nc.tensor.matmul(
                    z[1][1][1][:1, :256], scratch[:, :1], scratch,
                    start=True, stop=True,
                )

        def mm(t, s, cs=(0, 1)):
            p = PROC[t]
            if t == 0:
                lhsT = t0a[:, s * D:(s + 1) * D]
                rhs = {0: t0a[:, AOFF:AOFF + CHUNK], 1: t0b}
            else:
                buf, u = adj_bufs[t]
                lhsT = buf[:, u, s * D:(s + 1) * D]
                rhs = {
                    c: buf[:, u, AOFF + c * CHUNK:AOFF + (c + 1) * CHUNK]
                    for c in cs
                }
            for c in cs:
                nc.tensor.matmul(
                    z[p // HALF][s][c],
                    lhsT,
                    rhs[c],
                    start=(p % HALF == 0),
                    stop=(p % HALF == HALF - 1),
                )

        def copy_z(h, s):
            # psum fp32 -> sbuf fp16, chunk 0 on vector / chunk 1 on scalar
            nc.vector.tensor_copy(out=zt[h][s][0], in_=z[h][s][0])
            nc.scalar.activation(out=zt[h][s][1], in_=z[h][s][1], func=AF.Copy)

        def wagg(h, s, start, stop):
            for c in range(NCH):
                nc.tensor.matmul(z[0][s][c], w_sb, zt[h][s][c], start=start, stop=stop)

        # ---- Main stream. Tile 0 runs chunk-0 of both branches first so it
        # only waits on the first (split) transfer.
        mm(0, 0, cs=(0,))
        mm(0, 1, cs=(0,))
        mm(0, 0, cs=(1,))
        mm(0, 1, cs=(1,))
        dummy_mm(2)
        for t in MAIN[1:]:
            for s in range(2):
                mm(t, s)
            if t == MAIN[1]:
                dummy_mm(2)
            elif t in (MAIN[2], MAIN[3], MAIN[4], MAIN[6], MAIN[8]):
                dummy_mm(1)
            if t == MAIN[HALF - 1]:
                for s in range(2):
                    copy_z(0, s)
            if t == MAIN[HALF + 15]:
                for s in range(2):
                    wagg(0, s, start=True, stop=False)

        # Tail (12 SBUF-resident tiles): see schedule comment up top.
        for t in TAIL:
            mm(t, 0)
        copy_z(1, 0)
        for t in TAIL[0:2]:
            mm(t, 1)
        wagg(1, 0, start=False, stop=True)
        for c in range(NCH):
            nc.scalar.activation(
                out=h_sb[0][c],
                in_=z[0][0][c],
                func=AF.Relu,
                bias=b_sb,
                accum_out=csum[c],
            )
        nc.vector.tensor_add(out=csum_tot, in0=csum[0], in1=csum[1])
        nc.scalar.activation(
            out=c_sb, in_=csum_tot, func=AF.Sigmoid, scale=1.0 / (CS * ADJ_SCALE)
        )
        # 9 branch-1 matmul tiles fully cover the ~4.5 us zt-copy -> wagg ->
        # relu+sum x2 -> sigmoid -> cw chain so the PE never idles into a
        # clock re-throttle right before the score matmuls.
        for t in TAIL[2:11]:
            mm(t, 1)
        cw_ps = z[1][0][0]
        nc.tensor.matmul(cw_ps[:, :1], wbt_sb, c_sb, start=True, stop=True)
        nc.vector.tensor_copy(out=cw_sb, in_=cw_ps[:, :1])
        # branch-0 scores into banks freed by the branch-0 copies/relu
        sc0 = [z[1][0][1], z[0][0][0]]
        for c in range(NCH):
            nc.tensor.matmul(sc0[c][:1, :], cw_sb, h_sb[0][c], start=True, stop=True)
        # branch-0 scores to sbuf and out to DRAM mid-stream (bb is added on
        # the host, so these are plain copies)
        nc.vector.tensor_copy(out=out_sb[:, 0, 0:CHUNK], in_=sc0[0][:1, :])
        nc.scalar.activation(
            out=out_sb[:, 0, CHUNK:], in_=sc0[1][:1, :], func=AF.Copy
        )
        nc.scalar.dma_start(out=out[0:1, :].unsqueeze(0), in_=out_sb[:, 0:1])
        for t in TAIL[11:14]:
            mm(t, 1, cs=(0,))
        nc.vector.tensor_copy(out=zt[1][1][0], in_=z[1][1][0])
        for t in TAIL[11:14]:
            mm(t, 1, cs=(1,))
        nc.tensor.matmul(z[0][1][0], w_sb, zt[1][1][0], start=False, stop=True)
        nc.scalar.activation(out=zt[1][1][1], in_=z[1][1][1], func=AF.Copy)
        nc.vector.tensor_scalar(
            out=h_sb[1][0],
            in0=z[0][1][0],
            scalar1=b_sb,
            scalar2=0.0,
            op0=mybir.AluOpType.add,
            op1=mybir.AluOpType.max,
        )
        nc.tensor.matmul(z[0][1][1], w_sb, zt[1][1][1], start=False, stop=True)
        sc1 = [z[1][1][0], z[1][1][1]]
        nc.tensor.matmul(sc1[0][:1, :], cw_sb, h_sb[1][0], start=True, stop=True)
        nc.scalar.activation(
            out=h_sb[1][1], in_=z[0][1][1], func=AF.Relu, bias=b_sb
        )
        nc.vector.tensor_copy(out=out_sb[:, 1, 0:CHUNK], in_=sc1[0][:1, :])
        nc.tensor.matmul(sc1[1][:1, :], cw_sb, h_sb[1][1], start=True, stop=True)
        nc.scalar.activation(
            out=out_sb[:, 1, CHUNK:], in_=sc1[1][:1, :], func=AF.Copy
        )
        nc.scalar.dma_start(out=out[1:2, :].unsqueeze(0), in_=out_sb[:, 1:2])


_MODULE_CACHE: list = []


def get_module() -> bass.Bass:
    if not _MODULE_CACHE:
        _MODULE_CACHE.append(_build_module())
    return _MODULE_CACHE[0]


def shard_inputs(inputs: dict) -> list[dict]:
    """Full inputs -> per-core input maps (row-block sharding of adjT).

    comb[p, t, :] = [seq1[128t+p, :] | seq2[128t+p, :] | adjT rows] (f16),
    adj pre-scaled by 256; wbt = Wb.T/256; bvec = 256*b (see module doc).
    """
    s1 = np.asarray(inputs["seq1"], np.float32)[0].astype(np.float16)
    s2 = np.asarray(inputs["seq2"], np.float32)[0].astype(np.float16)
    seq_part = np.stack([s1, s2], axis=0).reshape(2, MT, 128, D).transpose(2, 1, 0, 3)
    seq_part = seq_part.reshape(128, MT, 2 * D)
    adj16 = (np.asarray(inputs["adj"], np.float32)[0] * ADJ_SCALE).astype(np.float16)
    w = np.asarray(inputs["W"], np.float32).astype(np.float16)
    wbt = (np.asarray(inputs["Wb"], np.float32).T / ADJ_SCALE).astype(np.float16)
    wq = np.ascontiguousarray(np.concatenate([w, wbt], axis=1))
    bvec = (np.asarray(inputs["b"], np.float32) * ADJ_SCALE).reshape(H, 1).copy()

    in_maps = []
    for k in range(NC):
        adjt = adj16[k * CS:(k + 1) * CS, :].T.reshape(MT, 128, CS).transpose(1, 0, 2)
        comb = np.empty((128, MT, ROW), np.float16)
        comb[:, :, :2 * D] = seq_part
        comb[:, :, 2 * D:] = adjt
        in_maps.append(
            {
                "comb": comb,
                "wq": wq,
                "bvec": bvec,
            }
        )
    return in_maps


def gather_output(
    core_outs: list[np.ndarray], cc_label: np.ndarray, bb: float = 0.0
) -> np.ndarray:
    """Per-core [2, CS] score blocks -> full [1, 2N] output.

    Scatter through cc_label mirrors the reference's .at[flat].set: entry
    (community k, position j) is the score of node cc_label[k, j]. The
    scalar bb is added here (exact, and off the device's critical path).
    """
    sc1 = np.concatenate([o[0] for o in core_outs]).astype(np.float32) + bb
    sc2 = np.concatenate([o[1] for o in core_outs]).astype(np.float32) + bb
    flat = np.asarray(cc_label).reshape(-1)
    ret1 = np.zeros(N, np.float32)
    ret2 = np.zeros(N, np.float32)
    ret1[flat] = sc1
    ret2[flat] = sc2
    return np.concatenate([ret1, ret2])[None, :]


def kernel(**inputs) -> np.ndarray:
    nc = get_module()
    in_maps = shard_inputs(inputs)
    res = run_bass_kernel_spmd(nc, in_maps, core_ids=list(range(NC)))
    core_outs = [res.results[k]["out"] for k in range(NC)]
    return gather_output(core_outs, inputs["cc_label"], float(inputs["bb"]))


if __name__ == "__main__":
    nc = get_module()
    print("module built ok")



# revision 39
# speedup vs baseline: 1.0439x; 1.0439x over previous
"""DGI (Deep Graph Infomax) forward kernel for 8 TRN2 NeuronCores.

Problem (all shapes hardcoded):
  seq1, seq2: [1, 8192, 128] f32   node features
  adj:        [1, 8192, 8192] f32  dense adjacency
  cc_label:   [8, 1024] i32        community partition (arange layout)
  W: [128,128], b: [128], Wb: [128,128], bb: [] f32
  out:        [1, 16384] f32       = concat(ret1, ret2)

Math per GCN branch: h = relu(adj @ (seq @ W) + b), reassociated to
(adj @ seq) @ W so the big contraction uses seq tiles as the stationary
operand and a host-transposed adj block as the moving operand. Everything
lives in "transposed" space (features on partitions): the community mean
is a free-axis accumulation and the bilinear scores are 1-column matmuls.

Sharding: core k owns nodes [1024k, 1024k+1024) == community k (cc_label
is arange). No collectives.

Data layout (the whole point of this version): seq tile t is consumed by
exactly the two matmuls of m-tile t, so the host packs, per partition p
and tile t, [seq1 row | seq2 row | adjT rows] into one combined tensor
comb[128, 64, 1280] f16 (2560 B per partition-tile). One DMA stream in
tile order IS the consumption order, every transfer has >=5 KB contiguous
per-partition runs (small runs were the previous bottleneck: 512 B
packets move at ~20 GB/s/engine vs 4 KB at full rate), and no persistent
seq buffer is needed.

Schedule:
  - sync HWDGE queue: comb[0:2][2:4][4:6][6:8][8:12][12:16] into a warm
    tile (fine-grained so the PE starts as early as possible), then
    8-tile groups [16:24][32:40][48:56][56:64] from a 3-buffer pool.
  - scalar HWDGE queue: params, then groups [24:32][40:48] (it starts
    ~1 us later than sync; interleaving two queues overlaps the
    small-packet warmup with steady streaming).
  - All small matmuls (W-contraction, cw, scores) in fp16. adj is
    pre-scaled by 256 (fp16 range); the relu computes h'=relu(z+256b)
    and the 1/256 is folded into host-side wbt and the sigmoid scale.
  - m-dim split in halves of 32 tiles; half-1's W-contraction + copies
    run mid-stream. For the last 16 m-tiles, branch 0 (which gates
    sigmoid -> cw -> all scores) streams before branch 1, and branch 0's
    epilogue PE ops are interleaved between branch-1 matmul batches (the
    PE queue is in-order). Branch 1 finishes c-major over the last 3
    tiles so chunk 0's epilogue overlaps chunk 1's matmuls.
  - Output via a single DMA on the sync queue.
"""

import numpy as np

import concourse.bass as bass
import concourse.tile as tile
from concourse import bacc, mybir
from concourse.bass_utils import run_bass_kernel_spmd

N = 8192          # nodes
D = 128           # input feature dim
H = 128           # hidden dim
NC = 8            # communities / cores
CS = N // NC      # community size (nodes per core)
MT = N // 128     # number of 128-row m-tiles (64)
HALF = MT // 2
CHUNK = 512       # matmul moving free dim (psum bank width in fp32)
NCH = CS // CHUNK # n-chunks per core (2)

F32 = mybir.dt.float32
F16 = mybir.dt.float16
ADJ_SCALE = 256.0  # keeps fp16(adj*scale) in the normal range; undone via
                   # host-prescaled wbt (scores) and the sigmoid scale (mean)

ROW = 2 * D + CS   # per-partition-tile row: seq1 | seq2 | adjT = 1280 f16
AOFF = 2 * D       # adj offset within a row

# Stream/processing architecture (lessons from five traced schedules):
#  - The main stream rides ONE HWDGE queue (sync), strictly in consumption
#    order. Two queues racing over the stream starve the PE: HBM bandwidth
#    goes to whichever queue has descriptors in flight, so the queue
#    holding the PE's next tile can lag many tiles behind.
#  - The m-tiles processed LAST (the "tail", where branch 0 finishes early
#    to gate sigmoid -> cw -> scores) are remapped to tiles 2..13, which
#    the scalar queue delivers during the ramp. The dual-queue ramp is
#    additive (the early bottleneck is per-queue descriptor pipelines, not
#    HBM), the scalar queue stops pulling by ~16 us (no mid-stream race),
#    and the 2x-rate tail sweep plus the whole epilogue runs out of SBUF -
#    also insulating the finish from stream-end straggler DMA engines
#    (observed: one engine finishing its packet backlog 10 us after the
#    other 15, gating the last group's completion semaphore).
#  - Tile 0 is split at the chunk-0/chunk-1 adj boundary so the first
#    matmuls start before its chunk-1 columns land; the next few stream
#    ranges stay small to bound PE wait quantization during the ramp.
MAIN = list(range(50))                 # streamed + processed in order
TAIL = list(range(50, MT))             # 14 m-tiles processed last
SYNC_RANGES = [(1, 2), (2, 3), (3, 4), (4, 6), (6, 8), (8, 10), (10, 12),
               (12, 14), (14, 16), (16, 18), (18, 20), (20, 22), (22, 24),
               (24, 26), (26, 28), (28, 30), (30, 32), (32, 34), (34, 36),
               (36, 38), (38, 40), (40, 42), (42, 44), (44, 46), (46, 48),
               (48, 50), (50, 52), (52, 54), (54, 56), (56, 58), (58, 60),
               (60, 62), (62, 63), (63, 64)]
SCALAR_RANGES = []
PROC = {t: i for i, t in enumerate(MAIN + TAIL)}  # processing index

# PE clock warmup: the HAM clock gate holds the PE at 1.2 GHz until it has
# been busy ~3.4 us, so the first real matmuls would otherwise run at half
# rate. While the first comb tiles are still in flight, run dummy 256-col
# matmuls on a memset scratch tile (no DMA dependency): they start right
# after the engine preamble (~7.2 us) and warm the clock to 2.4 GHz by the
# time real data lands. 256 cols cold = ~213 ns each, so a late-arriving
# real matmul is delayed by at most one dummy.
N_WARM_MM = 13

def _build_module() -> bass.Bass:
    nc = bacc.Bacc()

    comb = nc.declare_dram_parameter("comb", [128, MT, ROW], F16, isOutput=False)
    wq = nc.declare_dram_parameter("wq", [D, 2 * H], F16, isOutput=False)
    bvec = nc.declare_dram_parameter("bvec", [H, 1], F32, isOutput=False)
    out = nc.declare_dram_parameter("out", [2, CS], F32, isOutput=True)

    with tile.TileContext(nc) as tc:
        _emit(tc, comb, wq, bvec, out)
    nc.finalize()
    return nc


def _emit(tc, comb, wq, bvec, out):
    nc = tc.nc
    AF = mybir.ActivationFunctionType
    with (
        tc.tile_pool(name="singles", bufs=1) as singles,
        tc.tile_pool(name="adj_pool", bufs=1) as adj_pool,
        tc.tile_pool(name="misc", bufs=1) as misc,
        tc.tile_pool(name="psum", bufs=1, space="PSUM") as psum,
    ):
        # ---- Params on the scalar HWDGE queue; warmup scratch via gpsimd.
        scratch = singles.tile([128, 256], F16)
        nc.gpsimd.memset(scratch, 0.0)
        # Trigger the lazy sigmoid ACT_TABLE_LOAD (table_sel=1, ~1.3 us)
        # now, in the startup shadow - at first real use it lands on the
        # sigmoid -> cw -> scores critical path and idles the PE long
        # enough to re-throttle its clock.
        sig_warm = singles.tile([1, 1], F32)
        nc.scalar.activation(out=sig_warm, in_=scratch[:1, :1], func=AF.Sigmoid)
        wq_sb = singles.tile([D, 2 * H], F16)
        nc.scalar.dma_start(out=wq_sb, in_=wq[:])
        b_sb = singles.tile([H, 1], F32)
        nc.scalar.dma_start(out=b_sb, in_=bvec[:])
        w_sb = wq_sb[:, 0:H]
        wbt_sb = wq_sb[:, H:2 * H]

        # Every range gets its own buffer: with a rotating pool the DMA of
        # group i+k gates on consumption of group i, which lock-steps the
        # stream behind the PE. 64 tiles of comb fit in SBUF outright.
        # Tile 0 is split at the chunk-0/chunk-1 adj boundary (two separate
        # tiles so the chunk-0 matmuls wait only on the first transfer).
        t0a = adj_pool.tile([128, AOFF + CHUNK], F16)
        nc.sync.dma_start(out=t0a, in_=comb[:, 0, :AOFF + CHUNK])
        t0b = adj_pool.tile([128, CHUNK], F16)
        nc.sync.dma_start(out=t0b, in_=comb[:, 0, AOFF + CHUNK:])
        adj_bufs = {}
        for q_ranges, eng in ((SYNC_RANGES, nc.sync), (SCALAR_RANGES, nc.scalar)):
            for t0, t1 in q_ranges:
                gn = t1 - t0
                buf = adj_pool.tile([128, gn, ROW], F16, name=f"adj_g{t0}")
                eng.dma_start(out=buf, in_=comb[:, t0:t1])
                for u in range(gn):
                    adj_bufs[t0 + u] = (buf, u)

        # ---- Tiles. One psum tile (bank) per (half, branch, chunk):
        # PSUM is bank-major, so a multi-bank tile cannot be addressed as
        # per-partition-contiguous and trips psum bookkeeping.
        z_sep = {
            (h, s): [
                psum.tile([128, CHUNK], F32, name=f"z_{h}_{s}_{c}")
                for c in range(NCH)
            ]
            for h, s in ((0, 0), (0, 1), (1, 0), (1, 1))
        }

        def zap(h, s, c):
            return z_sep[(h, s)][c]
        # zt is reused across the two m-halves (WAR dep on the half-1
        # W-contraction orders the half-2 copies automatically)
        zt_sc = [
            misc.tile([128, NCH, CHUNK], F16, name=f"zt_{s}") for s in range(2)
        ]
        zt = [zt_sc, zt_sc]
        # h reuses the zt buffers: zt[s][c]'s last reader is the half-2
        # W-contraction, which precedes the relu that writes h (WAR dep)
        h_sb = zt_sc
        csum = [misc.tile([H, 1], F32, name=f"csum_{c}") for c in range(NCH)]
        csum_s = misc.tile([H, 1], F32)
        c_sb = misc.tile([H, 1], F16)
        cw_sb = misc.tile([H, 1], F16)
        out_sb = misc.tile([1, 2, CS], F32)

        def dummy_mm(n=1, bank=None):
            # HAM-insurance filler: keeps the PE busy through supply or
            # dependency waits so the clock gate never re-throttles; costs
            # 213 ns each if the awaited data was on time. The target bank
            # must have no open accumulation group at that point (a start
            # inside an open group corrupts it); its next real matmul
            # restarts its group, overwriting the garbage.
            if bank is None:
                bank = z_sep[(1, 1)][1]
            for _ in range(n):
                nc.tensor.matmul(
                    bank[:1, :256], scratch[:, :1], scratch,
                    start=True, stop=True,
                )

        # ---- PE clock warmup (see N_WARM_MM comment).
        dummy_mm(N_WARM_MM)

        def mm(t, s, cs=(0, 1)):
            p = PROC[t]
            if t == 0:
                lhsT = t0a[:, s * D:(s + 1) * D]
                rhs = {0: t0a[:, AOFF:AOFF + CHUNK], 1: t0b}
            else:
                buf, u = adj_bufs[t]
                lhsT = buf[:, u, s * D:(s + 1) * D]
                rhs = {
                    c: buf[:, u, AOFF + c * CHUNK:AOFF + (c + 1) * CHUNK]
                    for c in cs
                }
            for c in cs:
                nc.tensor.matmul(
                    zap(p // HALF, s, c),
                    lhsT,
                    rhs[c],
                    start=(p % HALF == 0),
                    stop=(p % HALF == HALF - 1),
                )

        def copy_z(h, s):
            # psum fp32 -> sbuf fp16, chunk 0 on vector / chunk 1 on scalar
            nc.vector.tensor_copy(out=zt[h][s][:, 0], in_=zap(h, s, 0))
            nc.scalar.activation(out=zt[h][s][:, 1], in_=zap(h, s, 1), func=AF.Copy)

        def wagg(h, s, start, stop):
            for c in range(NCH):
                nc.tensor.matmul(zap(0, s, c), w_sb, zt[h][s][:, c], start=start, stop=stop)

        # ---- Main stream. Tile 0 runs chunk-0 of both branches first so it
        # only waits on the first (split) transfer.
        mm(0, 0, cs=(0,))
        mm(0, 1, cs=(0,))
        mm(0, 0, cs=(1,))
        mm(0, 1, cs=(1,))
        dummy_mm(2)
        for t in MAIN[1:]:
            for s in range(2):
                mm(t, s)
            if t == MAIN[1]:
                dummy_mm(2)
            elif t in (MAIN[2], MAIN[3], MAIN[4], MAIN[6], MAIN[8]):
                dummy_mm(1)
            if t == MAIN[HALF - 1]:
                for s in range(2):
                    copy_z(0, s)
            if t == MAIN[HALF + 15]:
                for s in range(2):
                    wagg(0, s, start=True, stop=False)

        # Tail (12 SBUF-resident tiles): see schedule comment up top.
        for t in TAIL:
            mm(t, 0)
        copy_z(1, 0)
        for t in TAIL[0:2]:
            mm(t, 1)
        wagg(1, 0, start=False, stop=True)
        # Entire relu -> community-sum -> sigmoid chain on the scalar
        # engine: csum[1] is folded into the sigmoid via its (unscaled)
        # bias operand, avoiding a vector-engine add and two cross-engine
        # semaphore hops (~0.4 us each).
        for c in range(NCH):
            nc.scalar.activation(
                out=h_sb[0][:, c],
                in_=zap(0, 0, c),
                func=AF.Relu,
                bias=b_sb,
                accum_out=csum[c],
            )
        nc.scalar.activation(
            out=csum_s, in_=csum[1], func=AF.Copy, scale=1.0 / (CS * ADJ_SCALE)
        )
        nc.scalar.activation(
            out=c_sb,
            in_=csum[0],
            func=AF.Sigmoid,
            scale=1.0 / (CS * ADJ_SCALE),
            bias=csum_s,
        )
        # 9 branch-1 matmul tiles fully cover the ~4.5 us zt-copy -> wagg ->
        # relu+sum x2 -> sigmoid -> cw chain so the PE never idles into a
        # clock re-throttle right before the score matmuls.
        for t in TAIL[2:11]:
            mm(t, 1)
        # dummy filler: if the sigmoid chain is still in flight, keep the
        # PE busy so the clock gate stays at 2.4 GHz for the score matmuls.
        # Targets the (closed) cw bank - only its column 0 is read, after
        # cw's start=True overwrites it.
        dummy_mm(3, bank=z_sep[(1, 0)][0])
        cw_ps = z_sep[(1, 0)][0]
        nc.tensor.matmul(cw_ps[:, :1], wbt_sb, c_sb, start=True, stop=True)
        nc.vector.tensor_copy(out=cw_sb, in_=cw_ps[:, :1])
        # branch-0 scores into banks freed by the branch-0 copies/relu
        sc0 = [z_sep[(1, 0)][1], z_sep[(0, 0)][0]]
        for c in range(NCH):
            nc.tensor.matmul(sc0[c][:1, :], cw_sb, h_sb[0][:, c], start=True, stop=True)
        # branch-0 scores to sbuf and out to DRAM mid-stream (bb is added on
        # the host, so these are plain copies)
        nc.vector.tensor_copy(out=out_sb[:, 0, 0:CHUNK], in_=sc0[0][:1, :])
        nc.scalar.activation(
            out=out_sb[:, 0, CHUNK:], in_=sc0[1][:1, :], func=AF.Copy
        )
        nc.scalar.dma_start(out=out[0:1, :].unsqueeze(0), in_=out_sb[:, 0:1])
        for t in TAIL[11:14]:
            mm(t, 1, cs=(0,))
        nc.vector.tensor_copy(out=zt[1][1][:, 0], in_=z_sep[(1, 1)][0])
        for t in TAIL[11:14]:
            mm(t, 1, cs=(1,))
        nc.tensor.matmul(z_sep[(0, 1)][0], w_sb, zt[1][1][:, 0], start=False, stop=True)
        nc.scalar.activation(out=zt[1][1][:, 1], in_=z_sep[(1, 1)][1], func=AF.Copy)
        nc.vector.tensor_scalar(
            out=h_sb[1][:, 0],
            in0=z_sep[(0, 1)][0],
            scalar1=b_sb,
            scalar2=0.0,
            op0=mybir.AluOpType.add,
            op1=mybir.AluOpType.max,
        )
        nc.tensor.matmul(z_sep[(0, 1)][1], w_sb, zt[1][1][:, 1], start=False, stop=True)
        sc1 = [z_sep[(1, 1)][0], z_sep[(1, 1)][1]]
        nc.tensor.matmul(sc1[0][:1, :], cw_sb, h_sb[1][:, 0], start=True, stop=True)
        nc.scalar.activation(
            out=h_sb[1][:, 1], in_=z_sep[(0, 1)][1], func=AF.Relu, bias=b_sb
        )
        nc.vector.tensor_copy(out=out_sb[:, 1, 0:CHUNK], in_=sc1[0][:1, :])
        nc.tensor.matmul(sc1[1][:1, :], cw_sb, h_sb[1][:, 1], start=True, stop=True)
        nc.scalar.activation(
            out=out_sb[:, 1, CHUNK:], in_=sc1[1][:1, :], func=AF.Copy
        )
        nc.scalar.dma_start(out=out[1:2, :].unsqueeze(0), in_=out_sb[:, 1:2])


_MODULE_CACHE: list = []


def get_module() -> bass.Bass:
    if not _MODULE_CACHE:
        _MODULE_CACHE.append(_build_module())
    return _MODULE_CACHE[0]


def shard_inputs(inputs: dict) -> list[dict]:
    """Full inputs -> per-core input maps (row-block sharding of adjT).

    comb[p, t, :] = [seq1[128t+p, :] | seq2[128t+p, :] | adjT rows] (f16),
    adj pre-scaled by 256; wbt = Wb.T/256; bvec = 256*b (see module doc).
    """
    s1 = np.asarray(inputs["seq1"], np.float32)[0].astype(np.float16)
    s2 = np.asarray(inputs["seq2"], np.float32)[0].astype(np.float16)
    seq_part = np.stack([s1, s2], axis=0).reshape(2, MT, 128, D).transpose(2, 1, 0, 3)
    seq_part = seq_part.reshape(128, MT, 2 * D)
    adj16 = (np.asarray(inputs["adj"], np.float32)[0] * ADJ_SCALE).astype(np.float16)
    w = np.asarray(inputs["W"], np.float32).astype(np.float16)
    wbt = (np.asarray(inputs["Wb"], np.float32).T / ADJ_SCALE).astype(np.float16)
    wq = np.ascontiguousarray(np.concatenate([w, wbt], axis=1))
    bvec = (np.asarray(inputs["b"], np.float32) * ADJ_SCALE).reshape(H, 1).copy()

    in_maps = []
    for k in range(NC):
        adjt = adj16[k * CS:(k + 1) * CS, :].T.reshape(MT, 128, CS).transpose(1, 0, 2)
        comb = np.empty((128, MT, ROW), np.float16)
        comb[:, :, :2 * D] = seq_part
        comb[:, :, 2 * D:] = adjt
        in_maps.append(
            {
                "comb": comb,
                "wq": wq,
                "bvec": bvec,
            }
        )
    return in_maps


def gather_output(
    core_outs: list[np.ndarray], cc_label: np.ndarray, bb: float = 0.0
) -> np.ndarray:
    """Per-core [2, CS] score blocks -> full [1, 2N] output.

    Scatter through cc_label mirrors the reference's .at[flat].set: entry
    (community k, position j) is the score of node cc_label[k, j]. The
    scalar bb is added here (exact, and off the device's critical path).
    """
    sc1 = np.concatenate([o[0] for o in core_outs]).astype(np.float32) + bb
    sc2 = np.concatenate([o[1] for o in core_outs]).astype(np.float32) + bb
    flat = np.asarray(cc_label).reshape(-1)
    ret1 = np.zeros(N, np.float32)
    ret2 = np.zeros(N, np.float32)
    ret1[flat] = sc1
    ret2[flat] = sc2
    return np.concatenate([ret1, ret2])[None, :]


def kernel(**inputs) -> np.ndarray:
    nc = get_module()
    in_maps = shard_inputs(inputs)
    res = run_bass_kernel_spmd(nc, in_maps, core_ids=list(range(NC)))
    core_outs = [res.results[k]["out"] for k in range(NC)]
    return gather_output(core_outs, inputs["cc_label"], float(inputs["bb"]))


if __name__ == "__main__":
    nc = get_module()
    print("module built ok")



# revision 40
# speedup vs baseline: 1.0511x; 1.0070x over previous
"""DGI (Deep Graph Infomax) forward kernel for 8 TRN2 NeuronCores.

Problem (all shapes hardcoded):
  seq1, seq2: [1, 8192, 128] f32   node features
  adj:        [1, 8192, 8192] f32  dense adjacency
  cc_label:   [8, 1024] i32        community partition (arange layout)
  W: [128,128], b: [128], Wb: [128,128], bb: [] f32
  out:        [1, 16384] f32       = concat(ret1, ret2)

Math per GCN branch: h = relu(adj @ (seq @ W) + b), reassociated to
(adj @ seq) @ W so the big contraction uses seq tiles as the stationary
operand and a host-transposed adj block as the moving operand. Everything
lives in "transposed" space (features on partitions): the community mean
is a free-axis accumulation and the bilinear scores are 1-column matmuls.

Sharding: core k owns nodes [1024k, 1024k+1024) == community k (cc_label
is arange). No collectives.

Data layout (the whole point of this version): seq tile t is consumed by
exactly the two matmuls of m-tile t, so the host packs, per partition p
and tile t, [seq1 row | seq2 row | adjT rows] into one combined tensor
comb[128, 64, 1280] f16 (2560 B per partition-tile). One DMA stream in
tile order IS the consumption order, every transfer has >=5 KB contiguous
per-partition runs (small runs were the previous bottleneck: 512 B
packets move at ~20 GB/s/engine vs 4 KB at full rate), and no persistent
seq buffer is needed.

Schedule:
  - sync HWDGE queue: comb[0:2][2:4][4:6][6:8][8:12][12:16] into a warm
    tile (fine-grained so the PE starts as early as possible), then
    8-tile groups [16:24][32:40][48:56][56:64] from a 3-buffer pool.
  - scalar HWDGE queue: params, then groups [24:32][40:48] (it starts
    ~1 us later than sync; interleaving two queues overlaps the
    small-packet warmup with steady streaming).
  - All small matmuls (W-contraction, cw, scores) in fp16. adj is
    pre-scaled by 256 (fp16 range); the relu computes h'=relu(z+256b)
    and the 1/256 is folded into host-side wbt and the sigmoid scale.
  - m-dim split in halves of 32 tiles; half-1's W-contraction + copies
    run mid-stream. For the last 16 m-tiles, branch 0 (which gates
    sigmoid -> cw -> all scores) streams before branch 1, and branch 0's
    epilogue PE ops are interleaved between branch-1 matmul batches (the
    PE queue is in-order). Branch 1 finishes c-major over the last 3
    tiles so chunk 0's epilogue overlaps chunk 1's matmuls.
  - Output via a single DMA on the sync queue.
"""

import numpy as np

import concourse.bass as bass
import concourse.tile as tile
from concourse import bacc, mybir
from concourse.bass_utils import run_bass_kernel_spmd

N = 8192          # nodes
D = 128           # input feature dim
H = 128           # hidden dim
NC = 8            # communities / cores
CS = N // NC      # community size (nodes per core)
MT = N // 128     # number of 128-row m-tiles (64)
HALF = MT // 2
CHUNK = 512       # matmul moving free dim (psum bank width in fp32)
NCH = CS // CHUNK # n-chunks per core (2)

F32 = mybir.dt.float32
F16 = mybir.dt.float16
ADJ_SCALE = 256.0  # keeps fp16(adj*scale) in the normal range; undone via
                   # host-prescaled wbt (scores) and the sigmoid scale (mean)

ROW = 2 * D + CS   # per-partition-tile row: seq1 | seq2 | adjT = 1280 f16
AOFF = 2 * D       # adj offset within a row

# Stream/processing architecture (lessons from five traced schedules):
#  - The main stream rides ONE HWDGE queue (sync), strictly in consumption
#    order. Two queues racing over the stream starve the PE: HBM bandwidth
#    goes to whichever queue has descriptors in flight, so the queue
#    holding the PE's next tile can lag many tiles behind.
#  - The m-tiles processed LAST (the "tail", where branch 0 finishes early
#    to gate sigmoid -> cw -> scores) are remapped to tiles 2..13, which
#    the scalar queue delivers during the ramp. The dual-queue ramp is
#    additive (the early bottleneck is per-queue descriptor pipelines, not
#    HBM), the scalar queue stops pulling by ~16 us (no mid-stream race),
#    and the 2x-rate tail sweep plus the whole epilogue runs out of SBUF -
#    also insulating the finish from stream-end straggler DMA engines
#    (observed: one engine finishing its packet backlog 10 us after the
#    other 15, gating the last group's completion semaphore).
#  - Tile 0 is split at the chunk-0/chunk-1 adj boundary so the first
#    matmuls start before its chunk-1 columns land; the next few stream
#    ranges stay small to bound PE wait quantization during the ramp.
MAIN = list(range(50))                 # streamed + processed in order
TAIL = list(range(50, MT))             # 14 m-tiles processed last
SYNC_RANGES = [(1, 2), (2, 3), (3, 4), (4, 6), (6, 8), (8, 10), (10, 12),
               (12, 14), (14, 16), (16, 18), (18, 20), (20, 22), (22, 24),
               (24, 26), (26, 28), (28, 30), (30, 32), (32, 34), (34, 36),
               (36, 38), (38, 40), (40, 42), (42, 44), (44, 46), (46, 48),
               (48, 50), (50, 52), (52, 54), (54, 56), (56, 58), (58, 60),
               (60, 62), (62, 63), (63, 64)]
SCALAR_RANGES = []
PROC = {t: i for i, t in enumerate(MAIN + TAIL)}  # processing index

# PE clock warmup: the HAM clock gate holds the PE at 1.2 GHz until it has
# been busy ~3.4 us, so the first real matmuls would otherwise run at half
# rate. While the first comb tiles are still in flight, run dummy 256-col
# matmuls on a memset scratch tile (no DMA dependency): they start right
# after the engine preamble (~7.2 us) and warm the clock to 2.4 GHz by the
# time real data lands. 256 cols cold = ~213 ns each, so a late-arriving
# real matmul is delayed by at most one dummy.
N_WARM_MM = 13

def _build_module() -> bass.Bass:
    nc = bacc.Bacc()

    comb = nc.declare_dram_parameter("comb", [128, MT, ROW], F16, isOutput=False)
    wq = nc.declare_dram_parameter("wq", [D, 2 * H], F16, isOutput=False)
    bvec = nc.declare_dram_parameter("bvec", [H, 1], F32, isOutput=False)
    out = nc.declare_dram_parameter("out", [2, CS], F32, isOutput=True)

    with tile.TileContext(nc) as tc:
        _emit(tc, comb, wq, bvec, out)
    nc.finalize()
    return nc


def _emit(tc, comb, wq, bvec, out):
    nc = tc.nc
    AF = mybir.ActivationFunctionType
    with (
        tc.tile_pool(name="singles", bufs=1) as singles,
        tc.tile_pool(name="adj_pool", bufs=1) as adj_pool,
        tc.tile_pool(name="misc", bufs=1) as misc,
        tc.tile_pool(name="psum", bufs=1, space="PSUM") as psum,
    ):
        # ---- Params on the scalar HWDGE queue; warmup scratch via gpsimd.
        scratch = singles.tile([128, 256], F16)
        nc.gpsimd.memset(scratch, 0.0)
        # Trigger the lazy sigmoid ACT_TABLE_LOAD (table_sel=1, ~1.3 us)
        # now, in the startup shadow - at first real use it lands on the
        # sigmoid -> cw -> scores critical path and idles the PE long
        # enough to re-throttle its clock.
        sig_warm = singles.tile([1, 1], F32)
        nc.scalar.activation(out=sig_warm, in_=scratch[:1, :1], func=AF.Sigmoid)
        wq_sb = singles.tile([D, 2 * H], F16)
        nc.scalar.dma_start(out=wq_sb, in_=wq[:])
        b_sb = singles.tile([H, 1], F32)
        nc.scalar.dma_start(out=b_sb, in_=bvec[:])
        w_sb = wq_sb[:, 0:H]
        wbt_sb = wq_sb[:, H:2 * H]

        # Every range gets its own buffer: with a rotating pool the DMA of
        # group i+k gates on consumption of group i, which lock-steps the
        # stream behind the PE. 64 tiles of comb fit in SBUF outright.
        # Tile 0 is split at the chunk-0/chunk-1 adj boundary (two separate
        # tiles so the chunk-0 matmuls wait only on the first transfer).
        t0a = adj_pool.tile([128, AOFF + CHUNK], F16)
        nc.sync.dma_start(out=t0a, in_=comb[:, 0, :AOFF + CHUNK])
        t0b = adj_pool.tile([128, CHUNK], F16)
        nc.sync.dma_start(out=t0b, in_=comb[:, 0, AOFF + CHUNK:])
        adj_bufs = {}
        for q_ranges, eng in ((SYNC_RANGES, nc.sync), (SCALAR_RANGES, nc.scalar)):
            for t0, t1 in q_ranges:
                gn = t1 - t0
                buf = adj_pool.tile([128, gn, ROW], F16, name=f"adj_g{t0}")
                eng.dma_start(out=buf, in_=comb[:, t0:t1])
                for u in range(gn):
                    adj_bufs[t0 + u] = (buf, u)

        # ---- Tiles. One psum tile (bank) per (half, branch, chunk):
        # PSUM is bank-major, so a multi-bank tile cannot be addressed as
        # per-partition-contiguous and trips psum bookkeeping.
        z_sep = {
            (h, s): [
                psum.tile([128, CHUNK], F32, name=f"z_{h}_{s}_{c}")
                for c in range(NCH)
            ]
            for h, s in ((0, 0), (0, 1), (1, 0), (1, 1))
        }

        def zap(h, s, c):
            return z_sep[(h, s)][c]
        # zt is reused across the two m-halves (WAR dep on the half-1
        # W-contraction orders the half-2 copies automatically)
        zt_sc = [
            misc.tile([128, NCH, CHUNK], F16, name=f"zt_{s}") for s in range(2)
        ]
        zt = [zt_sc, zt_sc]
        # h reuses the zt buffers: zt[s][c]'s last reader is the half-2
        # W-contraction, which precedes the relu that writes h (WAR dep)
        h_sb = zt_sc
        csum = [misc.tile([H, 1], F32, name=f"csum_{c}") for c in range(NCH)]
        csum_s = misc.tile([H, 1], F32)
        c_sb = misc.tile([H, 1], F16)
        cw_sb = misc.tile([H, 1], F16)
        out_sb = misc.tile([1, 2, CS], F32)

        def dummy_mm(n=1, bank=None):
            # HAM-insurance filler: keeps the PE busy through supply or
            # dependency waits so the clock gate never re-throttles; costs
            # 213 ns each if the awaited data was on time. The target bank
            # must have no open accumulation group at that point (a start
            # inside an open group corrupts it); its next real matmul
            # restarts its group, overwriting the garbage.
            if bank is None:
                bank = z_sep[(1, 1)][1]
            for _ in range(n):
                nc.tensor.matmul(
                    bank[:1, :256], scratch[:, :1], scratch,
                    start=True, stop=True,
                )

        # ---- PE clock warmup (see N_WARM_MM comment).
        dummy_mm(N_WARM_MM)

        def mm(t, s, cs=(0, 1)):
            p = PROC[t]
            if t == 0:
                lhsT = t0a[:, s * D:(s + 1) * D]
                rhs = {0: t0a[:, AOFF:AOFF + CHUNK], 1: t0b}
            else:
                buf, u = adj_bufs[t]
                lhsT = buf[:, u, s * D:(s + 1) * D]
                rhs = {
                    c: buf[:, u, AOFF + c * CHUNK:AOFF + (c + 1) * CHUNK]
                    for c in cs
                }
            for c in cs:
                nc.tensor.matmul(
                    zap(p // HALF, s, c),
                    lhsT,
                    rhs[c],
                    start=(p % HALF == 0),
                    stop=(p % HALF == HALF - 1),
                )

        def copy_z(h, s):
            # psum fp32 -> sbuf fp16, chunk 0 on vector / chunk 1 on scalar
            nc.vector.tensor_copy(out=zt[h][s][:, 0], in_=zap(h, s, 0))
            nc.scalar.activation(out=zt[h][s][:, 1], in_=zap(h, s, 1), func=AF.Copy)

        def wagg(h, s, start, stop):
            for c in range(NCH):
                nc.tensor.matmul(zap(0, s, c), w_sb, zt[h][s][:, c], start=start, stop=stop)

        # ---- Main stream. Tile 0 runs chunk-0 of both branches first so it
        # only waits on the first (split) transfer.
        mm(0, 0, cs=(0,))
        mm(0, 1, cs=(0,))
        mm(0, 0, cs=(1,))
        mm(0, 1, cs=(1,))
        dummy_mm(2)
        for t in MAIN[1:]:
            for s in range(2):
                mm(t, s)
            if t == MAIN[1]:
                dummy_mm(2)
            elif t in (MAIN[2], MAIN[3], MAIN[4], MAIN[6], MAIN[8]):
                dummy_mm(1)
            if t == MAIN[HALF - 1]:
                for s in range(2):
                    copy_z(0, s)
            if t == MAIN[HALF + 15]:
                for s in range(2):
                    wagg(0, s, start=True, stop=False)

        # Tail (12 SBUF-resident tiles): see schedule comment up top.
        for t in TAIL:
            mm(t, 0)
        copy_z(1, 0)
        for t in TAIL[0:2]:
            mm(t, 1)
        wagg(1, 0, start=False, stop=True)
        # Entire relu -> community-sum -> sigmoid chain on the scalar
        # engine: csum[1] is folded into the sigmoid via its (unscaled)
        # bias operand, avoiding a vector-engine add and two cross-engine
        # semaphore hops (~0.4 us each).
        for c in range(NCH):
            nc.scalar.activation(
                out=h_sb[0][:, c],
                in_=zap(0, 0, c),
                func=AF.Relu,
                bias=b_sb,
                accum_out=csum[c],
            )
        nc.scalar.activation(
            out=csum_s, in_=csum[1], func=AF.Copy, scale=1.0 / (CS * ADJ_SCALE)
        )
        nc.scalar.activation(
            out=c_sb,
            in_=csum[0],
            func=AF.Sigmoid,
            scale=1.0 / (CS * ADJ_SCALE),
            bias=csum_s,
        )
        # 9 branch-1 matmul tiles fully cover the ~4.5 us zt-copy -> wagg ->
        # relu+sum x2 -> sigmoid -> cw chain so the PE never idles into a
        # clock re-throttle right before the score matmuls.
        for t in TAIL[2:11]:
            mm(t, 1)
        # Keep the PE clock warm across the relu->sigmoid chain: these
        # dummies READ the relu outputs, so the scheduler cannot hoist
        # them ahead of the chain - they execute exactly during the
        # remaining chain ops, and the score matmuls stay at 2.4 GHz.
        # They land in the (closed) cw bank; only its column 0 is read,
        # after cw's start=True overwrites it.
        for c in range(NCH):
            for _ in range(3):
                nc.tensor.matmul(
                    z_sep[(1, 0)][0][:1, :256], scratch[:, :1],
                    h_sb[0][:, c, :256], start=True, stop=True,
                )
        cw_ps = z_sep[(1, 0)][0]
        nc.tensor.matmul(cw_ps[:, :1], wbt_sb, c_sb, start=True, stop=True)
        nc.vector.tensor_copy(out=cw_sb, in_=cw_ps[:, :1])
        # branch-0 scores into banks freed by the branch-0 copies/relu
        sc0 = [z_sep[(1, 0)][1], z_sep[(0, 0)][0]]
        for c in range(NCH):
            nc.tensor.matmul(sc0[c][:1, :], cw_sb, h_sb[0][:, c], start=True, stop=True)
        # branch-0 scores to sbuf and out to DRAM mid-stream (bb is added on
        # the host, so these are plain copies)
        nc.vector.tensor_copy(out=out_sb[:, 0, 0:CHUNK], in_=sc0[0][:1, :])
        nc.scalar.activation(
            out=out_sb[:, 0, CHUNK:], in_=sc0[1][:1, :], func=AF.Copy
        )
        nc.scalar.dma_start(out=out[0:1, :].unsqueeze(0), in_=out_sb[:, 0:1])
        for t in TAIL[11:14]:
            mm(t, 1, cs=(0,))
        nc.vector.tensor_copy(out=zt[1][1][:, 0], in_=z_sep[(1, 1)][0])
        for t in TAIL[11:14]:
            mm(t, 1, cs=(1,))
        nc.tensor.matmul(z_sep[(0, 1)][0], w_sb, zt[1][1][:, 0], start=False, stop=True)
        nc.scalar.activation(out=zt[1][1][:, 1], in_=z_sep[(1, 1)][1], func=AF.Copy)
        nc.vector.tensor_scalar(
            out=h_sb[1][:, 0],
            in0=z_sep[(0, 1)][0],
            scalar1=b_sb,
            scalar2=0.0,
            op0=mybir.AluOpType.add,
            op1=mybir.AluOpType.max,
        )
        nc.tensor.matmul(z_sep[(0, 1)][1], w_sb, zt[1][1][:, 1], start=False, stop=True)
        sc1 = [z_sep[(1, 1)][0], z_sep[(1, 1)][1]]
        nc.tensor.matmul(sc1[0][:1, :], cw_sb, h_sb[1][:, 0], start=True, stop=True)
        nc.scalar.activation(
            out=h_sb[1][:, 1], in_=z_sep[(0, 1)][1], func=AF.Relu, bias=b_sb
        )
        nc.vector.tensor_copy(out=out_sb[:, 1, 0:CHUNK], in_=sc1[0][:1, :])
        nc.tensor.matmul(sc1[1][:1, :], cw_sb, h_sb[1][:, 1], start=True, stop=True)
        nc.scalar.activation(
            out=out_sb[:, 1, CHUNK:], in_=sc1[1][:1, :], func=AF.Copy
        )
        nc.scalar.dma_start(out=out[1:2, :].unsqueeze(0), in_=out_sb[:, 1:2])


_MODULE_CACHE: list = []


def get_module() -> bass.Bass:
    if not _MODULE_CACHE:
        _MODULE_CACHE.append(_build_module())
    return _MODULE_CACHE[0]


def shard_inputs(inputs: dict) -> list[dict]:
    """Full inputs -> per-core input maps (row-block sharding of adjT).

    comb[p, t, :] = [seq1[128t+p, :] | seq2[128t+p, :] | adjT rows] (f16),
    adj pre-scaled by 256; wbt = Wb.T/256; bvec = 256*b (see module doc).
    """
    s1 = np.asarray(inputs["seq1"], np.float32)[0].astype(np.float16)
    s2 = np.asarray(inputs["seq2"], np.float32)[0].astype(np.float16)
    seq_part = np.stack([s1, s2], axis=0).reshape(2, MT, 128, D).transpose(2, 1, 0, 3)
    seq_part = seq_part.reshape(128, MT, 2 * D)
    adj16 = (np.asarray(inputs["adj"], np.float32)[0] * ADJ_SCALE).astype(np.float16)
    w = np.asarray(inputs["W"], np.float32).astype(np.float16)
    wbt = (np.asarray(inputs["Wb"], np.float32).T / ADJ_SCALE).astype(np.float16)
    wq = np.ascontiguousarray(np.concatenate([w, wbt], axis=1))
    bvec = (np.asarray(inputs["b"], np.float32) * ADJ_SCALE).reshape(H, 1).copy()

    in_maps = []
    for k in range(NC):
        adjt = adj16[k * CS:(k + 1) * CS, :].T.reshape(MT, 128, CS).transpose(1, 0, 2)
        comb = np.empty((128, MT, ROW), np.float16)
        comb[:, :, :2 * D] = seq_part
        comb[:, :, 2 * D:] = adjt
        in_maps.append(
            {
                "comb": comb,
                "wq": wq,
                "bvec": bvec,
            }
        )
    return in_maps


def gather_output(
    core_outs: list[np.ndarray], cc_label: np.ndarray, bb: float = 0.0
) -> np.ndarray:
    """Per-core [2, CS] score blocks -> full [1, 2N] output.

    Scatter through cc_label mirrors the reference's .at[flat].set: entry
    (community k, position j) is the score of node cc_label[k, j]. The
    scalar bb is added here (exact, and off the device's critical path).
    """
    sc1 = np.concatenate([o[0] for o in core_outs]).astype(np.float32) + bb
    sc2 = np.concatenate([o[1] for o in core_outs]).astype(np.float32) + bb
    flat = np.asarray(cc_label).reshape(-1)
    ret1 = np.zeros(N, np.float32)
    ret2 = np.zeros(N, np.float32)
    ret1[flat] = sc1
    ret2[flat] = sc2
    return np.concatenate([ret1, ret2])[None, :]


def kernel(**inputs) -> np.ndarray:
    nc = get_module()
    in_maps = shard_inputs(inputs)
    res = run_bass_kernel_spmd(nc, in_maps, core_ids=list(range(NC)))
    core_outs = [res.results[k]["out"] for k in range(NC)]
    return gather_output(core_outs, inputs["cc_label"], float(inputs["bb"]))


if __name__ == "__main__":
    nc = get_module()
    print("module built ok")



# revision 41
# speedup vs baseline: 1.0520x; 1.0008x over previous
"""DGI (Deep Graph Infomax) forward kernel for 8 TRN2 NeuronCores.

Problem (all shapes hardcoded):
  seq1, seq2: [1, 8192, 128] f32   node features
  adj:        [1, 8192, 8192] f32  dense adjacency
  cc_label:   [8, 1024] i32        community partition (arange layout)
  W: [128,128], b: [128], Wb: [128,128], bb: [] f32
  out:        [1, 16384] f32       = concat(ret1, ret2)

Math per GCN branch: h = relu(adj @ (seq @ W) + b), reassociated to
(adj @ seq) @ W so the big contraction uses seq tiles as the stationary
operand and a host-transposed adj block as the moving operand. Everything
lives in "transposed" space (features on partitions): the community mean
is a free-axis accumulation and the bilinear scores are 1-column matmuls.

Sharding: core k owns nodes [1024k, 1024k+1024) == community k (cc_label
is arange). No collectives.

Data layout (the whole point of this version): seq tile t is consumed by
exactly the two matmuls of m-tile t, so the host packs, per partition p
and tile t, [seq1 row | seq2 row | adjT rows] into one combined tensor
comb[128, 64, 1280] f16 (2560 B per partition-tile). One DMA stream in
tile order IS the consumption order, every transfer has >=5 KB contiguous
per-partition runs (small runs were the previous bottleneck: 512 B
packets move at ~20 GB/s/engine vs 4 KB at full rate), and no persistent
seq buffer is needed.

Schedule:
  - sync HWDGE queue: comb[0:2][2:4][4:6][6:8][8:12][12:16] into a warm
    tile (fine-grained so the PE starts as early as possible), then
    8-tile groups [16:24][32:40][48:56][56:64] from a 3-buffer pool.
  - scalar HWDGE queue: params, then groups [24:32][40:48] (it starts
    ~1 us later than sync; interleaving two queues overlaps the
    small-packet warmup with steady streaming).
  - All small matmuls (W-contraction, cw, scores) in fp16. adj is
    pre-scaled by 256 (fp16 range); the relu computes h'=relu(z+256b)
    and the 1/256 is folded into host-side wbt and the sigmoid scale.
  - m-dim split in halves of 32 tiles; half-1's W-contraction + copies
    run mid-stream. For the last 16 m-tiles, branch 0 (which gates
    sigmoid -> cw -> all scores) streams before branch 1, and branch 0's
    epilogue PE ops are interleaved between branch-1 matmul batches (the
    PE queue is in-order). Branch 1 finishes c-major over the last 3
    tiles so chunk 0's epilogue overlaps chunk 1's matmuls.
  - Output via a single DMA on the sync queue.
"""

import numpy as np

import concourse.bass as bass
import concourse.tile as tile
from concourse import bacc, mybir
from concourse.bass_utils import run_bass_kernel_spmd

N = 8192          # nodes
D = 128           # input feature dim
H = 128           # hidden dim
NC = 8            # communities / cores
CS = N // NC      # community size (nodes per core)
MT = N // 128     # number of 128-row m-tiles (64)
HALF = MT // 2
CHUNK = 512       # matmul moving free dim (psum bank width in fp32)
NCH = CS // CHUNK # n-chunks per core (2)

F32 = mybir.dt.float32
F16 = mybir.dt.float16
ADJ_SCALE = 256.0  # keeps fp16(adj*scale) in the normal range; undone via
                   # host-prescaled wbt (scores) and the sigmoid scale (mean)

ROW = 2 * D + CS   # per-partition-tile row: seq1 | seq2 | adjT = 1280 f16
AOFF = 2 * D       # adj offset within a row

# Stream/processing architecture (lessons from five traced schedules):
#  - The main stream rides ONE HWDGE queue (sync), strictly in consumption
#    order. Two queues racing over the stream starve the PE: HBM bandwidth
#    goes to whichever queue has descriptors in flight, so the queue
#    holding the PE's next tile can lag many tiles behind.
#  - The m-tiles processed LAST (the "tail", where branch 0 finishes early
#    to gate sigmoid -> cw -> scores) are remapped to tiles 2..13, which
#    the scalar queue delivers during the ramp. The dual-queue ramp is
#    additive (the early bottleneck is per-queue descriptor pipelines, not
#    HBM), the scalar queue stops pulling by ~16 us (no mid-stream race),
#    and the 2x-rate tail sweep plus the whole epilogue runs out of SBUF -
#    also insulating the finish from stream-end straggler DMA engines
#    (observed: one engine finishing its packet backlog 10 us after the
#    other 15, gating the last group's completion semaphore).
#  - Tile 0 is split at the chunk-0/chunk-1 adj boundary so the first
#    matmuls start before its chunk-1 columns land; the next few stream
#    ranges stay small to bound PE wait quantization during the ramp.
MAIN = list(range(50))                 # streamed + processed in order
TAIL = list(range(50, MT))             # 14 m-tiles processed last
SYNC_RANGES = [(1, 2), (2, 3), (3, 4), (4, 6), (6, 8), (8, 10), (10, 12),
               (12, 14), (14, 16), (16, 18), (18, 20), (20, 22), (22, 24),
               (24, 26), (26, 28), (28, 30), (30, 32), (32, 34), (34, 36),
               (36, 38), (38, 40), (40, 42), (42, 44), (44, 46), (46, 48),
               (48, 50), (50, 52), (52, 54), (54, 56), (56, 58), (58, 60),
               (60, 62), (62, 63), (63, 64)]
SCALAR_RANGES = []
PROC = {t: i for i, t in enumerate(MAIN + TAIL)}  # processing index

# PE clock warmup: the HAM clock gate holds the PE at 1.2 GHz until it has
# been busy ~3.4 us, so the first real matmuls would otherwise run at half
# rate. While the first comb tiles are still in flight, run dummy 256-col
# matmuls on a memset scratch tile (no DMA dependency): they start right
# after the engine preamble (~7.2 us) and warm the clock to 2.4 GHz by the
# time real data lands. 256 cols cold = ~213 ns each, so a late-arriving
# real matmul is delayed by at most one dummy.
N_WARM_MM = 13

def _build_module() -> bass.Bass:
    nc = bacc.Bacc()

    comb = nc.declare_dram_parameter("comb", [128, MT, ROW], F16, isOutput=False)
    wq = nc.declare_dram_parameter("wq", [D, 2 * H], F16, isOutput=False)
    bvec = nc.declare_dram_parameter("bvec", [H, 1], F32, isOutput=False)
    out = nc.declare_dram_parameter("out", [2, CS], F32, isOutput=True)

    with tile.TileContext(nc) as tc:
        _emit(tc, comb, wq, bvec, out)
    nc.finalize()
    return nc


def _emit(tc, comb, wq, bvec, out):
    nc = tc.nc
    AF = mybir.ActivationFunctionType
    with (
        tc.tile_pool(name="singles", bufs=1) as singles,
        tc.tile_pool(name="adj_pool", bufs=1) as adj_pool,
        tc.tile_pool(name="misc", bufs=1) as misc,
        tc.tile_pool(name="psum", bufs=1, space="PSUM") as psum,
    ):
        # ---- Params on the scalar HWDGE queue; warmup scratch via gpsimd.
        scratch = singles.tile([128, 256], F16)
        nc.gpsimd.memset(scratch, 0.0)
        # Trigger the lazy sigmoid ACT_TABLE_LOAD (table_sel=1, ~1.3 us)
        # now, in the startup shadow - at first real use it lands on the
        # sigmoid -> cw -> scores critical path and idles the PE long
        # enough to re-throttle its clock.
        sig_warm = singles.tile([1, 1], F32)
        nc.scalar.activation(out=sig_warm, in_=scratch[:1, :1], func=AF.Sigmoid)
        wq_sb = singles.tile([D, 2 * H], F16)
        nc.scalar.dma_start(out=wq_sb, in_=wq[:])
        b_sb = singles.tile([H, 1], F32)
        nc.scalar.dma_start(out=b_sb, in_=bvec[:])
        w_sb = wq_sb[:, 0:H]
        wbt_sb = wq_sb[:, H:2 * H]

        # Every range gets its own buffer: with a rotating pool the DMA of
        # group i+k gates on consumption of group i, which lock-steps the
        # stream behind the PE. 64 tiles of comb fit in SBUF outright.
        # Tile 0 is split at the chunk-0/chunk-1 adj boundary (two separate
        # tiles so the chunk-0 matmuls wait only on the first transfer).
        t0a = adj_pool.tile([128, AOFF + CHUNK], F16)
        nc.sync.dma_start(out=t0a, in_=comb[:, 0, :AOFF + CHUNK])
        t0b = adj_pool.tile([128, CHUNK], F16)
        nc.sync.dma_start(out=t0b, in_=comb[:, 0, AOFF + CHUNK:])
        adj_bufs = {}
        for q_ranges, eng in ((SYNC_RANGES, nc.sync), (SCALAR_RANGES, nc.scalar)):
            for t0, t1 in q_ranges:
                gn = t1 - t0
                buf = adj_pool.tile([128, gn, ROW], F16, name=f"adj_g{t0}")
                eng.dma_start(out=buf, in_=comb[:, t0:t1])
                for u in range(gn):
                    adj_bufs[t0 + u] = (buf, u)

        # ---- Tiles. One psum tile (bank) per (half, branch, chunk):
        # PSUM is bank-major, so a multi-bank tile cannot be addressed as
        # per-partition-contiguous and trips psum bookkeeping.
        z_sep = {
            (h, s): [
                psum.tile([128, CHUNK], F32, name=f"z_{h}_{s}_{c}")
                for c in range(NCH)
            ]
            for h, s in ((0, 0), (0, 1), (1, 0), (1, 1))
        }

        def zap(h, s, c):
            return z_sep[(h, s)][c]
        # zt is reused across the two m-halves (WAR dep on the half-1
        # W-contraction orders the half-2 copies automatically)
        zt_sc = [
            misc.tile([128, NCH, CHUNK], F16, name=f"zt_{s}") for s in range(2)
        ]
        zt = [zt_sc, zt_sc]
        # h reuses the zt buffers: zt[s][c]'s last reader is the half-2
        # W-contraction, which precedes the relu that writes h (WAR dep)
        h_sb = zt_sc
        csum = [misc.tile([H, 1], F32, name=f"csum_{c}") for c in range(NCH)]
        csum_s = misc.tile([H, 1], F32)
        c_sb = misc.tile([H, 1], F16)
        cw_sb = misc.tile([H, 1], F16)
        out_sb = misc.tile([1, 2, CS], F32)

        def dummy_mm(n=1, bank=None):
            # HAM-insurance filler: keeps the PE busy through supply or
            # dependency waits so the clock gate never re-throttles; costs
            # 213 ns each if the awaited data was on time. The target bank
            # must have no open accumulation group at that point (a start
            # inside an open group corrupts it); its next real matmul
            # restarts its group, overwriting the garbage.
            if bank is None:
                bank = z_sep[(1, 1)][1]
            for _ in range(n):
                nc.tensor.matmul(
                    bank[:1, :256], scratch[:, :1], scratch,
                    start=True, stop=True,
                )

        # ---- PE clock warmup (see N_WARM_MM comment).
        dummy_mm(N_WARM_MM)

        def mm(t, s, cs=(0, 1)):
            p = PROC[t]
            if t == 0:
                lhsT = t0a[:, s * D:(s + 1) * D]
                rhs = {0: t0a[:, AOFF:AOFF + CHUNK], 1: t0b}
            else:
                buf, u = adj_bufs[t]
                lhsT = buf[:, u, s * D:(s + 1) * D]
                rhs = {
                    c: buf[:, u, AOFF + c * CHUNK:AOFF + (c + 1) * CHUNK]
                    for c in cs
                }
            for c in cs:
                nc.tensor.matmul(
                    zap(p // HALF, s, c),
                    lhsT,
                    rhs[c],
                    start=(p % HALF == 0),
                    stop=(p % HALF == HALF - 1),
                )

        def copy_z(h, s):
            # psum fp32 -> sbuf fp16, chunk 0 on vector / chunk 1 on scalar
            nc.vector.tensor_copy(out=zt[h][s][:, 0], in_=zap(h, s, 0))
            nc.scalar.activation(out=zt[h][s][:, 1], in_=zap(h, s, 1), func=AF.Copy)

        def wagg(h, s, start, stop):
            for c in range(NCH):
                nc.tensor.matmul(zap(0, s, c), w_sb, zt[h][s][:, c], start=start, stop=stop)

        # ---- Main stream. Tile 0 runs chunk-0 of both branches first so it
        # only waits on the first (split) transfer.
        mm(0, 0, cs=(0,))
        mm(0, 1, cs=(0,))
        mm(0, 0, cs=(1,))
        mm(0, 1, cs=(1,))
        dummy_mm(2)
        for t in MAIN[1:]:
            for s in range(2):
                mm(t, s)
            if t == MAIN[1]:
                dummy_mm(2)
            elif t in (MAIN[2], MAIN[3], MAIN[4], MAIN[6], MAIN[8]):
                dummy_mm(1)
            if t == MAIN[HALF - 1]:
                for s in range(2):
                    copy_z(0, s)
            if t == MAIN[HALF + 15]:
                for s in range(2):
                    wagg(0, s, start=True, stop=False)

        # Tail (12 SBUF-resident tiles): see schedule comment up top.
        for t in TAIL:
            mm(t, 0)
        copy_z(1, 0)
        for t in TAIL[0:2]:
            mm(t, 1)
        wagg(1, 0, start=False, stop=True)
        # Entire relu -> community-sum -> sigmoid chain on the scalar
        # engine: csum[1] is folded into the sigmoid via its (unscaled)
        # bias operand, avoiding a vector-engine add and two cross-engine
        # semaphore hops (~0.4 us each).
        for c in range(NCH):
            nc.scalar.activation(
                out=h_sb[0][:, c],
                in_=zap(0, 0, c),
                func=AF.Relu,
                bias=b_sb,
                accum_out=csum[c],
            )
        nc.scalar.activation(
            out=csum_s, in_=csum[1], func=AF.Copy, scale=1.0 / (CS * ADJ_SCALE)
        )
        nc.scalar.activation(
            out=c_sb,
            in_=csum[0],
            func=AF.Sigmoid,
            scale=1.0 / (CS * ADJ_SCALE),
            bias=csum_s,
        )
        # 9 branch-1 matmul tiles fully cover the ~4.5 us zt-copy -> wagg ->
        # relu+sum x2 -> sigmoid -> cw chain so the PE never idles into a
        # clock re-throttle right before the score matmuls.
        for t in TAIL[2:11]:
            mm(t, 1)
        # Keep the PE clock warm across the relu->sigmoid chain: these
        # dummies READ the relu outputs, so the scheduler cannot hoist
        # them ahead of the chain - they execute exactly during the
        # remaining chain ops, and the score matmuls stay at 2.4 GHz.
        # They land in the (closed) cw bank; only its column 0 is read,
        # after cw's start=True overwrites it.
        for c in range(NCH):
            for _ in range(3):
                nc.tensor.matmul(
                    z_sep[(1, 0)][0][:1, :256], scratch[:, :1],
                    h_sb[0][:, c, :256], start=True, stop=True,
                )
        cw_ps = z_sep[(1, 0)][0]
        nc.tensor.matmul(cw_ps[:, :1], wbt_sb, c_sb, start=True, stop=True)
        nc.vector.tensor_copy(out=cw_sb, in_=cw_ps[:, :1])
        # branch-0 scores into banks freed by the branch-0 copies/relu
        sc0 = [z_sep[(1, 0)][1], z_sep[(0, 0)][0]]
        for c in range(NCH):
            nc.tensor.matmul(sc0[c][:1, :], cw_sb, h_sb[0][:, c], start=True, stop=True)
        # branch-0 scores to sbuf and out to DRAM mid-stream (bb is added on
        # the host, so these are plain copies)
        nc.vector.tensor_copy(out=out_sb[:, 0, 0:CHUNK], in_=sc0[0][:1, :])
        nc.scalar.activation(
            out=out_sb[:, 0, CHUNK:], in_=sc0[1][:1, :], func=AF.Copy
        )
        nc.scalar.dma_start(out=out[0:1, :].unsqueeze(0), in_=out_sb[:, 0:1], max_dma_last_dim=CS)
        for t in TAIL[11:14]:
            mm(t, 1, cs=(0,))
        nc.vector.tensor_copy(out=zt[1][1][:, 0], in_=z_sep[(1, 1)][0])
        for t in TAIL[11:14]:
            mm(t, 1, cs=(1,))
        nc.tensor.matmul(z_sep[(0, 1)][0], w_sb, zt[1][1][:, 0], start=False, stop=True)
        nc.scalar.activation(out=zt[1][1][:, 1], in_=z_sep[(1, 1)][1], func=AF.Copy)
        nc.vector.tensor_scalar(
            out=h_sb[1][:, 0],
            in0=z_sep[(0, 1)][0],
            scalar1=b_sb,
            scalar2=0.0,
            op0=mybir.AluOpType.add,
            op1=mybir.AluOpType.max,
        )
        nc.tensor.matmul(z_sep[(0, 1)][1], w_sb, zt[1][1][:, 1], start=False, stop=True)
        sc1 = [z_sep[(1, 1)][0], z_sep[(1, 1)][1]]
        nc.tensor.matmul(sc1[0][:1, :], cw_sb, h_sb[1][:, 0], start=True, stop=True)
        nc.scalar.activation(
            out=h_sb[1][:, 1], in_=z_sep[(0, 1)][1], func=AF.Relu, bias=b_sb
        )
        nc.vector.tensor_copy(out=out_sb[:, 1, 0:CHUNK], in_=sc1[0][:1, :])
        nc.tensor.matmul(sc1[1][:1, :], cw_sb, h_sb[1][:, 1], start=True, stop=True)
        # final copy split across both engines - it is the last link
        # before the row-1 DMA
        nc.scalar.activation(
            out=out_sb[:, 1, CHUNK:CHUNK + CHUNK // 2],
            in_=sc1[1][:1, :CHUNK // 2],
            func=AF.Copy,
        )
        nc.vector.tensor_copy(
            out=out_sb[:, 1, CHUNK + CHUNK // 2:], in_=sc1[1][:1, CHUNK // 2:]
        )
        nc.scalar.dma_start(out=out[1:2, :].unsqueeze(0), in_=out_sb[:, 1:2], max_dma_last_dim=CS)


_MODULE_CACHE: list = []


def get_module() -> bass.Bass:
    if not _MODULE_CACHE:
        _MODULE_CACHE.append(_build_module())
    return _MODULE_CACHE[0]


def shard_inputs(inputs: dict) -> list[dict]:
    """Full inputs -> per-core input maps (row-block sharding of adjT).

    comb[p, t, :] = [seq1[128t+p, :] | seq2[128t+p, :] | adjT rows] (f16),
    adj pre-scaled by 256; wbt = Wb.T/256; bvec = 256*b (see module doc).
    """
    s1 = np.asarray(inputs["seq1"], np.float32)[0].astype(np.float16)
    s2 = np.asarray(inputs["seq2"], np.float32)[0].astype(np.float16)
    seq_part = np.stack([s1, s2], axis=0).reshape(2, MT, 128, D).transpose(2, 1, 0, 3)
    seq_part = seq_part.reshape(128, MT, 2 * D)
    adj16 = (np.asarray(inputs["adj"], np.float32)[0] * ADJ_SCALE).astype(np.float16)
    w = np.asarray(inputs["W"], np.float32).astype(np.float16)
    wbt = (np.asarray(inputs["Wb"], np.float32).T / ADJ_SCALE).astype(np.float16)
    wq = np.ascontiguousarray(np.concatenate([w, wbt], axis=1))
    bvec = (np.asarray(inputs["b"], np.float32) * ADJ_SCALE).reshape(H, 1).copy()

    in_maps = []
    for k in range(NC):
        adjt = adj16[k * CS:(k + 1) * CS, :].T.reshape(MT, 128, CS).transpose(1, 0, 2)
        comb = np.empty((128, MT, ROW), np.float16)
        comb[:, :, :2 * D] = seq_part
        comb[:, :, 2 * D:] = adjt
        in_maps.append(
            {
                "comb": comb,
                "wq": wq,
                "bvec": bvec,
            }
        )
    return in_maps


def gather_output(
    core_outs: list[np.ndarray], cc_label: np.ndarray, bb: float = 0.0
) -> np.ndarray:
    """Per-core [2, CS] score blocks -> full [1, 2N] output.

    Scatter through cc_label mirrors the reference's .at[flat].set: entry
    (community k, position j) is the score of node cc_label[k, j]. The
    scalar bb is added here (exact, and off the device's critical path).
    """
    sc1 = np.concatenate([o[0] for o in core_outs]).astype(np.float32) + bb
    sc2 = np.concatenate([o[1] for o in core_outs]).astype(np.float32) + bb
    flat = np.asarray(cc_label).reshape(-1)
    ret1 = np.zeros(N, np.float32)
    ret2 = np.zeros(N, np.float32)
    ret1[flat] = sc1
    ret2[flat] = sc2
    return np.concatenate([ret1, ret2])[None, :]


def kernel(**inputs) -> np.ndarray:
    nc = get_module()
    in_maps = shard_inputs(inputs)
    res = run_bass_kernel_spmd(nc, in_maps, core_ids=list(range(NC)))
    core_outs = [res.results[k]["out"] for k in range(NC)]
    return gather_output(core_outs, inputs["cc_label"], float(inputs["bb"]))


if __name__ == "__main__":
    nc = get_module()
    print("module built ok")

